# revision 1
# baseline (speedup 1.0000x reference)
"""Trainium2 Bass kernel for nn_MultiInfAffine.

Math (reference):
    mu_n = mus / ||mus||_D                          [L=6, D=16, K=64]
    t    = <x, mu_n>                                 per (l, n, k)
    d    = arccos(clip(t))
    cost = 0.5 d^2 + alpha
    mc_l = 0.1 * ln sum_k exp(-cost/0.1)
    F    = recurrence over l:  F = wv_l relu(F) + (1-wv_l) mc_l,  wv = exp(-ws^2)
    out  = 0.1 * ln(1 + exp(-10 F))

Device chain per element (branch-free nested half-angle; avoids arccos):
    v   = 1 + t (+delta)          -- folded into the inner-product matmul via an
                                     appended ones-dimension (contract = 17)
    c2  = sqrt(s5 * v)            -- = cos(d/2)            [ACT Sqrt]
    v2  = c2 + 1                  -- [DVE tensor_scalar 2x mode]
    q   = 1/v2                    -- [DVE custom reciprocal_approx_fast]
    m   = (v2 - 2) * q            -- = -tan^2(d/4)         [DVE scalar_tensor_tensor]
    r   = sqrt(-m)                -- = tan(d/4) in [0,1]   [ACT Sqrt]
    a   = arctan(r)               -- = d/4 in [0, pi/4]    [ACT Arctan, in-domain]
    E   = DErf(4*sqrt(5)*a)       -- = (2/sqrt(pi)) exp(-5 d^2)  [ACT] -> bf16
         (sim fallback: Square then Exp)
    S_l = sum_k w_k E_k           -- reduction matmul, weights carry
                                     e^{-10 alpha} (and sqrt(pi)/2 for DErf)
then a small tail (Ln + 6-step recurrence + smooth-min) on re-tiled data.

Layout: 128 SBUF partitions = 2 layers x 64 components ("plane" g covers layers
2g, 2g+1; 3 planes). Points stream along the free axis. ACT instructions are
chained in emission order (add_dep_helper) so activation-table loads stay at
~3 per block instead of per tile; post-c2 ACT passes batch all 3 planes in one
instruction.
"""

import numpy as np
import ml_dtypes

import concourse.bacc as bacc
import concourse.tile as tile
from concourse import mybir
from concourse.bass_utils import run_bass_kernel_spmd
from concourse.tile_rust import add_dep_helper

N, D, L, K = 250000, 16, 6, 64
NCORES = 8
NPC = N // NCORES  # 31250 true points per core

# tiling (per core)
SC = 992      # points per subtile (columns; matmul halves fit one PSUM bank)
NSUB = 4      # subtiles per block
NBLK = 8      # blocks
NPAD = SC * NSUB * NBLK  # 31744 padded points per core
T = NPAD // 128          # 248 point-columns in the tail layout

DELTA = 3e-7             # ones-row pad so v = 1 + t + DELTA > 0 under fp32 noise
S5 = 0.5 * (1.0 - 6e-7)  # sqrt scale keeping s5*v < 1 strictly
DERF_SCALE = 4.0 * np.sqrt(5.0)  # DErf(4*sqrt(5)*a) = 2/sqrt(pi) exp(-5 d^2)

F32 = mybir.dt.float32
F32R = mybir.dt.float32r
BF16 = mybir.dt.bfloat16
AF = mybir.ActivationFunctionType
ALU = mybir.AluOpType


class _ActChain:
    """Serialize ACT instructions in emission order so the scheduler cannot
    interleave activation-table sets across phases."""

    def __init__(self):
        self.last = None

    def __call__(self, inst):
        if self.last is not None:
            add_dep_helper(inst.ins, self.last.ins, sync=False,
                           reason="act phase order")
        self.last = inst
        return inst


def _build(nblk=NBLK, nsub=NSUB, sc=SC, wv=None, repeat=1, use_derf=True):
    """Build the per-core Bass program. wv: np.float32[L] = exp(-ws^2).
    repeat > 1 wraps the whole body in a HW loop (for timing; idempotent).
    use_derf=False switches to Square+Exp (CoreSim implements those)."""
    assert wv is not None
    npad = nblk * nsub * sc

    nc = bacc.Bacc()

    xst = nc.dram_tensor("xst", [D + 1, npad], F32R, kind="ExternalInput")
    mu = nc.dram_tensor("mu", [D + 1, 3, 128], F32R, kind="ExternalInput")
    ow = nc.dram_tensor("ow", [128, 3, 6], BF16, kind="ExternalInput")
    fout = nc.dram_tensor("fout", [npad], F32, kind="ExternalOutput")
    sd = nc.dram_tensor("sd", [6, npad], F32)  # staging for S (l-major)

    # recurrence constants
    A = [float(wv[l]) for l in range(L)]
    B = [float((1.0 - wv[l]) * 0.1) for l in range(L)]

    with tile.TileContext(nc) as tc:
        with (
            tc.tile_pool(name="singles", bufs=1) as singles,
            tc.tile_pool(name="xs", bufs=3) as xpool,
            tc.tile_pool(name="vpsum", bufs=3, space="PSUM") as vpool,
            tc.tile_pool(name="spsum", bufs=2, space="PSUM") as spool,
            tc.tile_pool(name="c2", bufs=8) as c2pool,
            tc.tile_pool(name="q", bufs=3) as qpool,
            tc.tile_pool(name="e", bufs=2) as epool,
            tc.tile_pool(name="rz", bufs=2) as rzpool,
            tc.tile_pool(name="tail", bufs=1) as tailpool,
        ):
            mu_sb = singles.tile([D + 1, 3, 128], F32R)
            nc.sync.dma_start(out=mu_sb[:], in_=mu[:])
            ow_sb = singles.tile([128, 3, 6], BF16)
            nc.sync.dma_start(out=ow_sb[:], in_=ow[:])

            args = (nc, tc, nblk, nsub, sc, A, B, use_derf,
                    xst, sd, fout, mu_sb, ow_sb,
                    xpool, vpool, spool, c2pool, qpool, epool,
                    rzpool, tailpool)
            if repeat > 1:
                with tc.For_i(0, repeat, 1):
                    _emit_body(*args)
            else:
                _emit_body(*args)

    nc.compile()
    return nc


def _emit_body(nc, tc, nblk, nsub, sc, A, B, use_derf,
               xst, sd, fout, mu_sb, ow_sb,
               xpool, vpool, spool, c2pool, qpool, epool,
               rzpool, tailpool):
    npad = nblk * nsub * sc
    t_cols = npad // 128
    h = sc // 2  # matmul half-width (one PSUM bank)
    act = _ActChain()

    rz_tiles = {}

    def emit_ph1(b):
        # matmul v, ACT Sqrt(c2) [sqrt set], DVE chain -> rz = -tan^2(d/4)
        rz = rzpool.tile([128, 3, nsub * sc], F32, tag="rz")
        rz_tiles[b] = rz
        for s in range(nsub):
            c0 = (b * nsub + s) * sc
            xs_t = xpool.tile([D + 1, sc], F32R, tag="xs")
            nc.sync.dma_start(out=xs_t[:], in_=xst[:, c0:c0 + sc])
            c2_ts = []
            for g in range(3):
                v_t = vpool.tile([128, sc], F32, tag="v")
                # split at the PSUM bank boundary (512 fp32)
                for c in range(0, sc, 512):
                    ce = min(c + 512, sc)
                    nc.tensor.matmul(v_t[:, c:ce], mu_sb[:, g, :],
                                     xs_t[:, c:ce])
                c2_t = c2pool.tile([128, sc], F32, tag="c2")
                act(nc.scalar.activation(c2_t[:], v_t[:], AF.Sqrt, scale=S5))
                c2_ts.append(c2_t)
            for g in range(3):
                c2_t = c2_ts[g]
                nc.vector.tensor_scalar_add(c2_t[:], c2_t[:], 1.0)
                q_t = qpool.tile([128, sc], F32, tag="q")
                nc.vector.reciprocal_approx_fast(out=q_t[:], in_=c2_t[:])
                nc.vector.scalar_tensor_tensor(
                    out=rz[:, g, s * sc:(s + 1) * sc],
                    in0=c2_t[:], scalar=2.0, in1=q_t[:],
                    op0=ALU.subtract, op1=ALU.mult,
                )

    def emit_r(b):
        # ACT Sqrt [sqrt set] -> r = tan(d/4); 3 planes x 2 subtiles per instr
        rz = rz_tiles[b]
        for s in range(0, nsub, 2):
            sl = rz[:, :, s * sc:(s + 2) * sc]
            act(nc.scalar.activation(sl, sl, AF.Sqrt, scale=-1.0))

    def emit_atan(b):
        # ACT Arctan [trig set] in place -> rz = d/4
        rz = rz_tiles[b]
        for s in range(0, nsub, 2):
            sl = rz[:, :, s * sc:(s + 2) * sc]
            act(nc.scalar.activation(sl, sl, AF.Arctan))

    def emit_efold(b):
        # E (bf16) [erf/exp set], reduce matmul, stage S
        rz = rz_tiles[b]
        if not use_derf:
            for s in range(0, nsub, 2):
                sl = rz[:, :, s * sc:(s + 2) * sc]
                act(nc.scalar.activation(sl, sl, AF.Square, scale=4.0))
        e_ts = {}
        for s in range(0, nsub, 2):
            sl = rz[:, :, s * sc:(s + 2) * sc]
            e_t = epool.tile([128, 3, 2 * sc], BF16, tag="e")
            if use_derf:
                act(nc.scalar.activation(e_t[:], sl, AF.Derivative_Erf,
                                         scale=DERF_SCALE))
            else:
                act(nc.scalar.activation(e_t[:], sl, AF.Exp, scale=-5.0))
            e_ts[s] = e_t
            e_ts[s + 1] = None
        for s in range(0, nsub, 2):
            e_t = e_ts[s]
            sv_t = qpool.tile([6, 2 * sc], F32, tag="sv")
            for quarter in range(4):
                s_t = spool.tile([6, h], F32, tag="s")
                for g in range(3):
                    nc.tensor.matmul(
                        s_t[:], ow_sb[:, g, :],
                        e_t[:, g, quarter * h:(quarter + 1) * h],
                        start=(g == 0), stop=(g == 2),
                    )
                nc.vector.tensor_copy(
                    sv_t[:, quarter * h:(quarter + 1) * h], s_t[:])
            c0 = (b * nsub + s) * sc
            nc.sync.dma_start(out=sd[:, c0:c0 + 2 * sc], in_=sv_t[:])
        del rz_tiles[b]

    # Tail buffer: point j lands on (p = j//T, t = j%T); block b covers
    # exactly partitions [pb*b, pb*(b+1)) since nsub*sc is a multiple of T.
    mc = tailpool.tile([128, 6, t_cols], F32)
    blk_cols = nsub * sc
    pb = blk_cols // t_cols
    assert pb * t_cols == blk_cols

    def emit_mc_load(b):
        c0 = b * blk_cols
        for l in range(L):
            nc.sync.dma_start(
                out=mc[pb * b:pb * (b + 1), l, :],
                in_=sd[l, c0:c0 + blk_cols].rearrange("(p t) -> p t", p=pb),
            )

    # Software-pipelined block schedule. Block b+1's c2 phase (sqrt set) is
    # emitted right after block b's r phase (also sqrt set — no table load),
    # giving the DVE chain a full trig+erf phase of lead time.
    emit_ph1(0)
    for b in range(nblk):
        emit_r(b)
        if b + 1 < nblk:
            emit_ph1(b + 1)
        emit_atan(b)
        emit_efold(b)
        emit_mc_load(b)

    # ---- tail: Ln, recurrence, smooth-min, store
    act(nc.scalar.activation(mc[:], mc[:], AF.Ln))
    for l in range(L):
        nc.vector.tensor_scalar_mul(mc[:, l, :], mc[:, l, :], B[l])
    f_t = tailpool.tile([128, t_cols], F32)
    nc.vector.tensor_copy(f_t[:], mc[:, 0, :])
    for l in range(1, L):
        nc.vector.tensor_scalar_max(f_t[:], f_t[:], 0.0)
        nc.vector.scalar_tensor_tensor(
            out=f_t[:], in0=f_t[:], scalar=A[l], in1=mc[:, l, :],
            op0=ALU.mult, op1=ALU.add,
        )
    act(nc.scalar.activation(f_t[:], f_t[:], AF.Exp, scale=-10.0))
    act(nc.scalar.activation(f_t[:], f_t[:], AF.Ln, bias=1.0))
    nc.vector.tensor_scalar_mul(f_t[:], f_t[:], 0.1)
    nc.sync.dma_start(
        out=fout[:].rearrange("(p t) -> p t", p=128), in_=f_t[:]
    )


def _host_prep(xs, mus, alphas, ws, npad_per_core=NPAD, ncores=NCORES,
               use_derf=True):
    """Returns (shared inputs dict, list of per-core xst arrays, wv)."""
    mus = np.asarray(mus, np.float32)
    alphas = np.asarray(alphas, np.float32)
    ws = np.asarray(ws, np.float32)
    xs = np.asarray(xs, np.float32)

    mu_n = mus / np.linalg.norm(mus, axis=1, keepdims=True)  # [L, D, K]
    # mu layout: [17, 3, 128]; column j of plane g is (layer 2g + j//64, k = j%64)
    mu_aug = np.zeros((D + 1, 3, 128), np.float32)
    for g in range(3):
        for half in range(2):
            layer = 2 * g + half
            mu_aug[:D, g, 64 * half:64 * half + 64] = mu_n[layer]
    mu_aug[D, :, :] = 1.0 + DELTA

    # reduction weights carry e^{-10 alpha} (+ sqrt(pi)/2 for the DErf factor)
    wfac = float(np.sqrt(np.pi) / 2.0) if use_derf else 1.0
    ow = np.zeros((128, 3, 6), np.float32)
    for g in range(3):
        for half in range(2):
            layer = 2 * g + half
            ow[64 * half:64 * half + 64, g, layer] = (
                wfac * np.exp(-10.0 * alphas[layer].astype(np.float64))
            ).astype(np.float32)
    ow = ow.astype(ml_dtypes.bfloat16)

    wv = np.exp(-ws.astype(np.float32) ** 2).astype(np.float32)

    n = xs.shape[0]
    per = n // ncores
    xst_list = []
    for c in range(ncores):
        shard = xs[c * per:(c + 1) * per]
        aug = np.ones((shard.shape[0], D + 1), np.float32)
        aug[:, :D] = shard
        pad = np.zeros((npad_per_core, D + 1), np.float32)
        pad[:, D] = 1.0  # pad points: x = 0 -> v = 1 + delta, harmless
        pad[:shard.shape[0]] = aug
        xst_list.append(np.ascontiguousarray(pad.T))  # [17, npad]
    return {"mu": mu_aug, "ow": ow}, xst_list, wv


def prepare(xs, mus, alphas, ws, repeat=1, use_derf=True):
    """Build the Bass program and per-core input maps."""
    shared, xst_list, wv = _host_prep(xs, mus, alphas, ws, use_derf=use_derf)
    nc = _build(wv=wv, repeat=repeat, use_derf=use_derf)
    in_maps = [dict(shared, xst=xst_list[c]) for c in range(NCORES)]
    return nc, in_maps


def kernel(xs, mus, alphas, ws, trace=False, tmpdir=None):
    nc, in_maps = prepare(xs, mus, alphas, ws)
    res = run_bass_kernel_spmd(
        nc, in_maps, core_ids=list(range(NCORES)), trace=trace, tmpdir=tmpdir
    )
    per = N // NCORES
    out = np.concatenate([res.results[c]["fout"][:per] for c in range(NCORES)])
    kernel.last_results = res
    return out.astype(np.float32)



# revision 2
# speedup vs baseline: 1.5127x; 1.5127x over previous
"""Trainium2 Bass kernel for nn_MultiInfAffine.

Math (reference):
    mu_n = mus / ||mus||_D                          [L=6, D=16, K=64]
    t    = <x, mu_n>                                 per (l, n, k)
    cost = 0.5 * arccos(t)^2 + alpha
    mc_l = 0.1 * ln sum_k exp(-cost/0.1)
    F    = recurrence over l:  F = wv_l relu(F) + (1-wv_l) mc_l,  wv = exp(-ws^2)
    out  = 0.1 * ln(1 + exp(-10 F))

Device chain per element (2 custom-DVE passes + 1 ACT pass):
    t    = matmul                                   [PE, fp32r, contract 16]
    r1   = (t^2 + a t + b)(t^2 + c t + d)           [DVE custom MIA_Q4_ANT]
    x6   = r1 * (t^2 + g t + h)                     [DVE custom MIA_SX_ANT]
    E    = DErf(c6 * x6) = 2/sqrt(pi) exp(-(c6 x6)^2)  -> bf16   [ACT]
    S_l  = sum_k w_k E_k                            [PE reduce matmul]
  where P6(t) = c6 (t^2+at+b)(t^2+ct+d)(t^2+gt+h) satisfies
    P6(t)^2 ~= 5*arccos(t)^2 + C    (C = 8; weighted fit, err ~2e-4 where
                                     a term can be within e^-25 of its
                                     point's dominant term)
  and w_k = sqrt(pi)/2 * exp(C - 10 alpha_k) makes w_k E_k ~= exp(-10 cost).
Then a small tail (Ln + 6-step recurrence + smooth-min) on re-tiled data.

Layout: 128 SBUF partitions = 2 layers x 64 components ("plane" g covers
layers 2g, 2g+1; 3 planes). Points stream along the free axis in subtiles of
512 (one PSUM bank). ACT uses a single table function (DErf) for the whole
main loop; no table swaps.
"""

import numpy as np
import ml_dtypes

import concourse.bacc as bacc
import concourse.tile as tile
from concourse import mybir
from concourse.bass_utils import run_bass_kernel_spmd

N, D, L, K = 250000, 16, 6, 64
NCORES = 8
NPC = N // NCORES  # 31250 true points per core

# tiling (per core)
SC = 512       # points per subtile (columns; one PSUM bank per plane)
NSUB = 62      # subtiles per core
NPAD = SC * NSUB  # 31744 padded points per core
T = NPAD // 128   # 248 point-columns in the tail layout

# P6(t) = C6 * (t^2 + QA t + QB)(t^2 + QC t + QD)(t^2 + QG t + QH)
# P6^2 ~= 5*arccos(t)^2 + CEXP  (weighted minimax fit, deg 6, see module doc)
CEXP = 8.0
QA, QB = 3.14061245107967, 5.038208318193999
QC, QD = -1.8293342680211058, 6.778190637720332
QG, QH = -5.608048417976159, 7.5822856951162105
C6 = 0.017416233472857055

F32 = mybir.dt.float32
F32R = mybir.dt.float32r
BF16 = mybir.dt.bfloat16
AF = mybir.ActivationFunctionType
ALU = mybir.AluOpType

# ---- custom DVE op registration (idempotent, at import) ------------------- #

def _register_ops():
    import concourse.dve_ops as dve_ops
    from concourse.dve_spec import (
        Spec, Src0, Src1, C0, C1, C2, C3, sq, lower,
        _spill_c3_to_src1, _has_src1,
    )
    from concourse.dve_uop import DveOpSpec

    def mk(name, body, reference):
        if name in dve_ops._SUB_OPCODE_FOR_NAME:
            return next(op for op in dve_ops.OPS if op.name == name)
        spec = Spec(body=body, reference=reference)
        row = dve_ops._CUSTOM_DVE_ROW_BASE + len(dve_ops.OPS)
        assert row < 0x20
        shas = {}
        for ver in ("v3", "v4"):
            shas[ver] = DveOpSpec(
                name=name, opcode=row, uops=lower(spec, ver=ver),
                rd1_en=_has_src1(spec),
            ).sha(ver)
        op = dve_ops.DveOp(name, spec, False, shas)
        dve_ops.OPS.append(op)
        dve_ops._SUB_OPCODE_FOR_NAME[name] = row
        dve_ops.CUSTOM_DVE_SPECS[name] = spec
        return op

    # out = (t^2 + C0 t + C1)(t^2 + C2 t + C3); C3 spilled to in1 [P,1]
    s = sq(Src0)
    q4_body = _spill_c3_to_src1(
        (s + (Src0 * C0 + C1)) * (s + (Src0 * C2 + C3)))

    def q4_ref(in0, in1, c0, c1, c2):
        ss = in0 * in0
        return (ss + in0 * c0 + c1) * (ss + in0 * c2 + in1)

    # out = in0 * (in1^2 + C0 in1 + C1)
    s1 = sq(Src1)
    sx_body = Src0 * (s1 + (Src1 * C0 + C1))

    def sx_ref(in0, in1, c0, c1, c2):
        return in0 * (in1 * in1 + in1 * c0 + c1)

    return mk("MIA_Q4_ANT", q4_body, q4_ref), mk("MIA_SX_ANT", sx_body, sx_ref)


MIA_Q4, MIA_SX = _register_ops()


def _build(nsub=NSUB, sc=SC, wv=None, repeat=1):
    """Build the per-core Bass program. wv: np.float32[L] = exp(-ws^2).
    repeat > 1 wraps the whole body in a HW loop (for timing; idempotent)."""
    assert wv is not None
    npad = nsub * sc

    nc = bacc.Bacc()

    xst = nc.dram_tensor("xst", [D, npad], F32R, kind="ExternalInput")
    mu = nc.dram_tensor("mu", [D, 3, 128], F32R, kind="ExternalInput")
    ow = nc.dram_tensor("ow", [128, 3, 6], BF16, kind="ExternalInput")
    fout = nc.dram_tensor("fout", [npad], F32, kind="ExternalOutput")
    sd = nc.dram_tensor("sd", [6, npad], F32)  # staging for S (l-major)

    # recurrence constants
    A = [float(wv[l]) for l in range(L)]
    B = [float((1.0 - wv[l]) * 0.1) for l in range(L)]

    with tile.TileContext(nc) as tc:
        with (
            tc.tile_pool(name="singles", bufs=1) as singles,
            tc.tile_pool(name="xs", bufs=3) as xpool,
            tc.tile_pool(name="vpsum", bufs=2, space="PSUM") as vpool,
            tc.tile_pool(name="spsum", bufs=2, space="PSUM") as spool,
            tc.tile_pool(name="r1", bufs=3) as r1pool,
            tc.tile_pool(name="x6", bufs=3) as x6pool,
            tc.tile_pool(name="e", bufs=3) as epool,
            tc.tile_pool(name="sv", bufs=3) as svpool,
            tc.tile_pool(name="tail", bufs=1) as tailpool,
        ):
            mu_sb = singles.tile([D, 3, 128], F32R)
            nc.sync.dma_start(out=mu_sb[:], in_=mu[:])
            ow_sb = singles.tile([128, 3, 6], BF16)
            nc.sync.dma_start(out=ow_sb[:], in_=ow[:])
            qd_sb = singles.tile([128, 1], F32)
            nc.vector.memset(qd_sb[:], float(QD))

            args = (nc, tc, nsub, sc, A, B, xst, sd, fout,
                    mu_sb, ow_sb, qd_sb,
                    xpool, vpool, spool, r1pool, x6pool, epool, svpool,
                    tailpool)
            if repeat > 1:
                with tc.For_i(0, repeat, 1):
                    _emit_body(*args)
            else:
                _emit_body(*args)

    nc.compile()
    return nc


def _emit_body(nc, tc, nsub, sc, A, B, xst, sd, fout,
               mu_sb, ow_sb, qd_sb,
               xpool, vpool, spool, r1pool, x6pool, epool, svpool, tailpool):
    npad = nsub * sc
    t_cols = npad // 128

    for sidx in range(nsub):
        c0 = sidx * sc
        xs_t = xpool.tile([D, sc], F32R, tag="xs")
        nc.sync.dma_start(out=xs_t[:], in_=xst[:, c0:c0 + sc])
        v_t = vpool.tile([128, 3, sc], F32, tag="v")
        for g in range(3):
            nc.tensor.matmul(v_t[:, g, :], mu_sb[:, g, :], xs_t[:])
        r1_t = r1pool.tile([128, 3, sc], F32, tag="r1")
        nc.vector._custom_dve(
            MIA_Q4, out=r1_t[:].rearrange("p a b -> p (a b)"),
            in0=v_t[:].rearrange("p a b -> p (a b)"),
            in1=qd_sb[:], s0=QA, s1=QB, imm2=QC,
        )
        x6_t = x6pool.tile([128, 3, sc], F32, tag="x6")
        nc.vector._custom_dve(
            MIA_SX, out=x6_t[:], in0=r1_t[:], in1=v_t[:], s0=QG, s1=QH,
        )
        e_t = epool.tile([128, 3, sc], BF16, tag="e")
        nc.scalar.activation(e_t[:], x6_t[:], AF.Derivative_Erf, scale=C6)
        s_t = spool.tile([6, sc], F32, tag="s")
        for g in range(3):
            nc.tensor.matmul(s_t[:], ow_sb[:, g, :], e_t[:, g, :],
                             start=(g == 0), stop=(g == 2))
        sv_t = svpool.tile([6, sc], F32, tag="sv")
        nc.scalar.activation(sv_t[:], s_t[:], AF.Copy)
        nc.sync.dma_start(out=sd[:, c0:c0 + sc], in_=sv_t[:])

    # ---- tail: Ln, recurrence, smooth-min, store
    mc = tailpool.tile([128, 6, t_cols], F32)
    for l in range(L):
        nc.sync.dma_start(
            out=mc[:, l, :],
            in_=sd[l, :].rearrange("(p t) -> p t", p=128),
        )
    nc.scalar.activation(mc[:], mc[:], AF.Ln)
    for l in range(L):
        nc.vector.tensor_scalar_mul(mc[:, l, :], mc[:, l, :], B[l])
    f_t = tailpool.tile([128, t_cols], F32)
    nc.vector.tensor_copy(f_t[:], mc[:, 0, :])
    for l in range(1, L):
        nc.vector.tensor_scalar_max(f_t[:], f_t[:], 0.0)
        nc.vector.scalar_tensor_tensor(
            out=f_t[:], in0=f_t[:], scalar=A[l], in1=mc[:, l, :],
            op0=ALU.mult, op1=ALU.add,
        )
    nc.scalar.activation(f_t[:], f_t[:], AF.Exp, scale=-10.0)
    nc.scalar.activation(f_t[:], f_t[:], AF.Ln, bias=1.0)
    nc.vector.tensor_scalar_mul(f_t[:], f_t[:], 0.1)
    nc.sync.dma_start(
        out=fout[:].rearrange("(p t) -> p t", p=128), in_=f_t[:]
    )


def _host_prep(xs, mus, alphas, ws, npad_per_core=NPAD, ncores=NCORES):
    """Returns (shared inputs dict, list of per-core xst arrays, wv)."""
    mus = np.asarray(mus, np.float32)
    alphas = np.asarray(alphas, np.float32)
    ws = np.asarray(ws, np.float32)
    xs = np.asarray(xs, np.float32)

    mu_n = mus / np.linalg.norm(mus, axis=1, keepdims=True)  # [L, D, K]
    # mu layout: [16, 3, 128]; column j of plane g is (layer 2g + j//64, k = j%64)
    mu_aug = np.zeros((D, 3, 128), np.float32)
    for g in range(3):
        for half in range(2):
            layer = 2 * g + half
            mu_aug[:, g, 64 * half:64 * half + 64] = mu_n[layer]

    # reduction weights: sqrt(pi)/2 * exp(CEXP - 10 alpha)
    ow = np.zeros((128, 3, 6), np.float32)
    for g in range(3):
        for half in range(2):
            layer = 2 * g + half
            ow[64 * half:64 * half + 64, g, layer] = (
                (np.sqrt(np.pi) / 2.0)
                * np.exp(CEXP - 10.0 * alphas[layer].astype(np.float64))
            ).astype(np.float32)
    ow = ow.astype(ml_dtypes.bfloat16)

    wv = np.exp(-ws.astype(np.float32) ** 2).astype(np.float32)

    n = xs.shape[0]
    per = n // ncores
    xst_list = []
    for c in range(ncores):
        shard = xs[c * per:(c + 1) * per]
        pad = np.zeros((npad_per_core, D), np.float32)
        pad[:shard.shape[0]] = shard  # pad points: x = 0 -> t = 0, harmless
        xst_list.append(np.ascontiguousarray(pad.T))  # [16, npad]
    return {"mu": mu_aug, "ow": ow}, xst_list, wv


def prepare(xs, mus, alphas, ws, repeat=1):
    """Build the Bass program and per-core input maps."""
    shared, xst_list, wv = _host_prep(xs, mus, alphas, ws)
    nc = _build(wv=wv, repeat=repeat)
    in_maps = [dict(shared, xst=xst_list[c]) for c in range(NCORES)]
    return nc, in_maps


def kernel(xs, mus, alphas, ws, trace=False, tmpdir=None):
    nc, in_maps = prepare(xs, mus, alphas, ws)
    res = run_bass_kernel_spmd(
        nc, in_maps, core_ids=list(range(NCORES)), trace=trace, tmpdir=tmpdir
    )
    per = N // NCORES
    out = np.concatenate([res.results[c]["fout"][:per] for c in range(NCORES)])
    kernel.last_results = res
    return out.astype(np.float32)


# revision 9
# speedup vs baseline: 2.1138x; 1.3974x over previous
"""Trainium2 Bass kernel for nn_MultiInfAffine.

Math (reference):
    mu_n = mus / ||mus||_D                          [L=6, D=16, K=64]
    t    = <x, mu_n>                                 per (l, n, k)
    cost = 0.5 * arccos(t)^2 + alpha
    mc_l = 0.1 * ln sum_k exp(-cost/0.1)
    F    = recurrence over l:  F = wv_l relu(F) + (1-wv_l) mc_l,  wv = exp(-ws^2)
    out  = 0.1 * ln(1 + exp(-10 F))

Device chain per element (1 custom-DVE pass + 1 ACT pass):
    v    = t - RHO            [PE matmul, fp32r, contract 17 via ones-row]
    q    = v (v^2 + QA v + QB)(v^2 + QD)            [DVE custom MIA_Q5_ANT]
    E    = DErf(KAP q + BET) = 2/sqrt(pi) exp(-(KAP q + BET)^2)  -> bf16 [ACT]
    S_l  = sum_k w_k E_k                            [PE reduce matmul]
  where M(t) = KAP (t-RHO)((t-RHO)^2+QA(t-RHO)+QB)((t-RHO)^2+QD) + BET is a
  quintic with M(t)^2 ~= 5*arccos(t)^2 + CEXP (weighted fit; exponent err
  <= 7.3e-4 wherever a term can be within e^-25 of its point's dominant
  term), and w_k = sqrt(pi)/2 * exp(CEXP - 10 alpha_k) makes
  w_k E_k ~= exp(-10 cost). Then a small tail (Ln + 6-step recurrence +
  smooth-min) on re-tiled data.

Layout: 128 SBUF partitions = 2 layers x 64 components ("plane" g covers
layers 2g, 2g+1; 3 planes). Points stream along the free axis in subtiles of
512 (one PSUM bank per plane). ACT runs only DErf + Copy in the main loop
(both in the erf_derivative table set; no table swaps).
"""

import numpy as np
import ml_dtypes

import concourse.bacc as bacc
import concourse.tile as tile
from concourse import mybir
from concourse.bass_utils import run_bass_kernel_spmd

N, D, L, K = 250000, 16, 6, 64
NCORES = 8
NPC = N // NCORES  # 31250 true points per core

# tiling (per core)
SC = 512       # points per subtile (columns; one PSUM bank per plane)
NSUB = 62      # subtiles per core
NPAD = SC * NSUB  # 31744 padded points per core
T = NPAD // 128   # 248 point-columns in the tail layout

# M(t) = KAP*v*(v^2+QA*v+QB)*(v^2+QD) + BET, v = t-RHO; M^2 ~= 5 arccos^2 + CEXP
CEXP = 8.0
RHO = 1.650216200888483
QA = 6.037997075652353
QB = 12.799226390484804
QD = 4.497047893926167
KAP = 0.046074945478925385
BET = -1.4581460932891892

F32 = mybir.dt.float32
F32R = mybir.dt.float32r
BF16 = mybir.dt.bfloat16
AF = mybir.ActivationFunctionType
ALU = mybir.AluOpType

# ---- custom DVE op registration (idempotent, at import) ------------------- #

def _register_ops():
    import concourse.dve_ops as dve_ops
    from concourse.dve_spec import (
        Spec, Src0, C0, C1, C3, sq, lower, _has_src1, _spill_c3_to_src1,
    )
    from concourse.dve_uop import DveOpSpec

    def mk(name, body, reference):
        if name in dve_ops._SUB_OPCODE_FOR_NAME:
            return next(op for op in dve_ops.OPS if op.name == name)
        spec = Spec(body=body, reference=reference)
        row = dve_ops._CUSTOM_DVE_ROW_BASE + len(dve_ops.OPS)
        assert row < 0x20
        shas = {}
        for ver in ("v3", "v4"):
            shas[ver] = DveOpSpec(
                name=name, opcode=row, uops=lower(spec, ver=ver),
                rd1_en=_has_src1(spec),
            ).sha(ver)
        op = dve_ops.DveOp(name, spec, False, shas)
        dve_ops.OPS.append(op)
        dve_ops._SUB_OPCODE_FOR_NAME[name] = row
        dve_ops.CUSTOM_DVE_SPECS[name] = spec
        return op

    # out = v(v^2 + C0 v + C1)(v^2 + d) with v = in0; d rides C3, spilled to
    # in1 [P,1] and latched at element 0 (a streaming Src1 read of a [P,1]
    # AP would exhaust the src1 stream and hang the engine)
    s = sq(Src0)
    q5_body = _spill_c3_to_src1(((s + (Src0 * C0 + C1)) * Src0) * (s + C3))

    def q5_ref(in0, in1, c0, c1, c2):
        ss = in0 * in0
        return ((ss + in0 * c0 + c1) * in0) * (ss + in1)

    return mk("MIA_Q5_ANT", q5_body, q5_ref)


MIA_Q5 = _register_ops()


def _build(nsub=NSUB, sc=SC, wv=None, repeat=1):
    """Build the per-core Bass program. wv: np.float32[L] = exp(-ws^2).
    repeat > 1 wraps the whole body in a HW loop (for timing; idempotent)."""
    assert wv is not None
    npad = nsub * sc

    nc = bacc.Bacc()

    xst = nc.dram_tensor("xst", [D + 1, npad], F32R, kind="ExternalInput")
    mu = nc.dram_tensor("mu", [D + 1, 3, 128], F32R, kind="ExternalInput")
    ow = nc.dram_tensor("ow", [128, 3, 6], BF16, kind="ExternalInput")
    fout = nc.dram_tensor("fout", [npad], F32, kind="ExternalOutput")
    sd = nc.dram_tensor("sd", [6, npad], F32)  # staging for S (l-major)

    # recurrence constants
    A = [float(wv[l]) for l in range(L)]
    B = [float((1.0 - wv[l]) * 0.1) for l in range(L)]

    with tile.TileContext(nc) as tc:
        with (
            tc.tile_pool(name="singles", bufs=1) as singles,
            tc.tile_pool(name="xs", bufs=3) as xpool,
            tc.tile_pool(name="vpsum", bufs=2, space="PSUM") as vpool,
            tc.tile_pool(name="spsum", bufs=2, space="PSUM") as spool,
            tc.tile_pool(name="q5", bufs=3) as q5pool,
            tc.tile_pool(name="e", bufs=3) as epool,
            tc.tile_pool(name="sv", bufs=3) as svpool,
            tc.tile_pool(name="tail", bufs=1) as tailpool,
        ):
            mu_sb = singles.tile([D + 1, 3, 128], F32R)
            nc.sync.dma_start(out=mu_sb[:], in_=mu[:])
            ow_sb = singles.tile([128, 3, 6], BF16)
            nc.sync.dma_start(out=ow_sb[:], in_=ow[:])
            qd_sb = singles.tile([128, 1], F32)
            nc.vector.memset(qd_sb[:], float(QD))
            bet_sb = singles.tile([128, 1], F32)
            nc.vector.memset(bet_sb[:], float(BET))

            args = (nc, tc, nsub, sc, A, B, xst, sd, fout,
                    mu_sb, ow_sb, qd_sb, bet_sb,
                    xpool, vpool, spool, q5pool, epool, svpool, tailpool)
            if repeat > 1:
                with tc.For_i(0, repeat, 1):
                    _emit_body(*args)
            else:
                _emit_body(*args)

    nc.compile()
    return nc


def _emit_body(nc, tc, nsub, sc, A, B, xst, sd, fout,
               mu_sb, ow_sb, qd_sb, bet_sb,
               xpool, vpool, spool, q5pool, epool, svpool, tailpool):
    npad = nsub * sc
    t_cols = npad // 128

    for sidx in range(nsub):
        c0 = sidx * sc
        xs_t = xpool.tile([D + 1, sc], F32R, tag="xs")
        nc.sync.dma_start(out=xs_t[:], in_=xst[:, c0:c0 + sc])
        v_t = vpool.tile([128, 3, sc], F32, tag="v")
        for g in range(3):
            nc.tensor.matmul(v_t[:, g, :], mu_sb[:, g, :], xs_t[:])
        q5_t = q5pool.tile([128, 3, sc], F32, tag="q5")
        nc.vector._custom_dve(
            MIA_Q5, out=q5_t[:], in0=v_t[:], in1=qd_sb[:], s0=QA, s1=QB,
        )
        e_t = epool.tile([128, 3, sc], BF16, tag="e")
        nc.scalar.activation(e_t[:], q5_t[:], AF.Derivative_Erf,
                             scale=KAP, bias=bet_sb[:])
        s_t = spool.tile([6, sc], F32, tag="s")
        for g in range(3):
            nc.tensor.matmul(s_t[:], ow_sb[:, g, :], e_t[:, g, :],
                             start=(g == 0), stop=(g == 2))
        sv_t = svpool.tile([6, sc], F32, tag="sv")
        nc.scalar.activation(sv_t[:], s_t[:], AF.Copy)
        nc.sync.dma_start(out=sd[:, c0:c0 + sc], in_=sv_t[:])

    # ---- tail: Ln, recurrence, smooth-min, store
    mc = tailpool.tile([128, 6, t_cols], F32)
    for l in range(L):
        nc.sync.dma_start(
            out=mc[:, l, :],
            in_=sd[l, :].rearrange("(p t) -> p t", p=128),
        )
    nc.scalar.activation(mc[:], mc[:], AF.Ln)
    for l in range(L):
        nc.vector.tensor_scalar_mul(mc[:, l, :], mc[:, l, :], B[l])
    f_t = tailpool.tile([128, t_cols], F32)
    nc.vector.tensor_copy(f_t[:], mc[:, 0, :])
    for l in range(1, L):
        nc.vector.tensor_scalar_max(f_t[:], f_t[:], 0.0)
        nc.vector.scalar_tensor_tensor(
            out=f_t[:], in0=f_t[:], scalar=A[l], in1=mc[:, l, :],
            op0=ALU.mult, op1=ALU.add,
        )
    nc.scalar.activation(f_t[:], f_t[:], AF.Exp, scale=-10.0)
    nc.scalar.activation(f_t[:], f_t[:], AF.Ln, bias=1.0)
    nc.vector.tensor_scalar_mul(f_t[:], f_t[:], 0.1)
    nc.sync.dma_start(
        out=fout[:].rearrange("(p t) -> p t", p=128), in_=f_t[:]
    )


def _host_prep(xs, mus, alphas, ws, npad_per_core=NPAD, ncores=NCORES):
    """Returns (shared inputs dict, list of per-core xst arrays, wv)."""
    mus = np.asarray(mus, np.float32)
    alphas = np.asarray(alphas, np.float32)
    ws = np.asarray(ws, np.float32)
    xs = np.asarray(xs, np.float32)

    mu_n = mus / np.linalg.norm(mus, axis=1, keepdims=True)  # [L, D, K]
    # mu layout: [17, 3, 128]; column j of plane g is (layer 2g + j//64, k = j%64)
    mu_aug = np.zeros((D + 1, 3, 128), np.float32)
    for g in range(3):
        for half in range(2):
            layer = 2 * g + half
            mu_aug[:D, g, 64 * half:64 * half + 64] = mu_n[layer]
    mu_aug[D, :, :] = -RHO  # ones-row coefficient: v = t - RHO

    # reduction weights: sqrt(pi)/2 * exp(CEXP - 10 alpha)
    ow = np.zeros((128, 3, 6), np.float32)
    for g in range(3):
        for half in range(2):
            layer = 2 * g + half
            ow[64 * half:64 * half + 64, g, layer] = (
                (np.sqrt(np.pi) / 2.0)
                * np.exp(CEXP - 10.0 * alphas[layer].astype(np.float64))
            ).astype(np.float32)
    ow = ow.astype(ml_dtypes.bfloat16)

    wv = np.exp(-ws.astype(np.float32) ** 2).astype(np.float32)

    n = xs.shape[0]
    per = n // ncores
    xst_list = []
    for c in range(ncores):
        shard = xs[c * per:(c + 1) * per]
        aug = np.ones((shard.shape[0], D + 1), np.float32)
        aug[:, :D] = shard
        pad = np.zeros((npad_per_core, D + 1), np.float32)
        pad[:, D] = 1.0  # pad points: x = 0 -> v = -RHO, harmless
        pad[:shard.shape[0]] = aug
        xst_list.append(np.ascontiguousarray(pad.T))  # [17, npad]
    return {"mu": mu_aug, "ow": ow}, xst_list, wv


def prepare(xs, mus, alphas, ws, repeat=1):
    """Build the Bass program and per-core input maps."""
    shared, xst_list, wv = _host_prep(xs, mus, alphas, ws)
    nc = _build(wv=wv, repeat=repeat)
    in_maps = [dict(shared, xst=xst_list[c]) for c in range(NCORES)]
    return nc, in_maps


def kernel(xs, mus, alphas, ws, trace=False, tmpdir=None):
    nc, in_maps = prepare(xs, mus, alphas, ws)
    res = run_bass_kernel_spmd(
        nc, in_maps, core_ids=list(range(NCORES)), trace=trace, tmpdir=tmpdir
    )
    per = N // NCORES
    out = np.concatenate([res.results[c]["fout"][:per] for c in range(NCORES)])
    kernel.last_results = res
    return out.astype(np.float32)
